# revision 81
# baseline (speedup 1.0000x reference)
"""ConvVMamba TRN2 Bass kernel (v2).

Sharding: data-parallel over batch. B=8 -> one image per NeuronCore, all
weights replicated, no collectives.

Per-core layout: channels on SBUF partitions (C=96), pixels on the free dim
(L=64*64=4096).

Major design points vs v1 (cost model: 500us -> 299us/core):
  - ACT function-table discipline: each table phase uses a single table
    function (Sqrt / Gelu / Sigmoid / Ln) so the compiler inserts ~8
    LoadActFuncSet ops instead of ~131 (each costs ~1.3us; this alone was
    168us of ACT busy time in v1 -- Exp and Ln first-match into different
    table sets, so Exp/Ln pairs thrash the table).
  - LayerNorm rstd via ACT Sqrt + DVE reciprocal_approx_fast (AF.Rsqrt is
    banned in bass for accuracy; Exp/Ln trick thrashes tables).
  - Selective scan (d_state=1, A=-1 fast path): dA = sigmoid(-z) directly
    (one ACT op) and delta = -ln(dA), instead of softplus via Exp+Ln plus
    another Exp. General-A fallback keeps an extra Exp.
  - B/C rows (d_state=1: single xdbl rows) broadcast to 96 partitions by
    the DMA engines via a 0-stride repeat AP; the scan elementwise chain
    (uB, bso=delta*u*B, scan, y=h*C) runs full-tile on DVE.
  - Depthwise 7x7 on fp8 with DoubleRow matmuls: two vertical taps per
    matmul. The image is padded to 80-byte row stride (DR ifmap k-tile
    step must be 16-aligned; delta=1 within-row pairs fail at runtime on
    HW) and the conv runs on 6-row x 80-col flat blocks (PSUM 480 <= 512);
    junk pad columns are dropped on evacuation. Weights are per-channel
    power-of-2 scaled into fp8 range and descaled in the evacuation.
  - Residual adds folded into PSUM: an identity f32r matmul preloads the
    residual into the accumulation group of fc2/out-proj, so the evac is
    a single ACT/DVE bias op (GPSIMD cannot read PSUM at all).
  - Engine balancing: squares and SBUF-only multiplies on GPSIMD/Pool
    (0.42 efficiency - only short ops), PSUM-reading elementwise on DVE,
    PSUM evacuations with scale+bias on ACT.
  - Emission interleaves the dwconv7 blocks with the ConvNeXt-MLP chunk
    work so ACT/DVE/Pool overlap the PE-heavy conv phase.
  - PSUM tag budget (8 banks): cv x2 (conv7/conv3/xdbl accumulators),
    gen x2 (mean/dt/ip/op matmuls), gen2 x2 (LN variance + fc2), f1 x2.
    Tag rotation is a serialization point: accumulators that contend in
    the same pipeline phase must live on different tags.
"""

import sys
import numpy as np

sys.path.insert(0, "/opt/trn_rl_repo")

import ml_dtypes  # noqa: E402
import concourse.bass as bass  # noqa: E402
import concourse.bacc as bacc  # noqa: E402
import concourse.mybir as mybir  # noqa: E402
from concourse import tile  # noqa: E402
from concourse.tile import add_dep_helper  # noqa: E402
from concourse.bass_utils import run_bass_kernel_spmd  # noqa: E402

F32 = mybir.dt.float32
F32R = mybir.dt.float32r
BF16 = mybir.dt.bfloat16
FP8 = mybir.dt.float8e4
AF = mybir.ActivationFunctionType
OP = mybir.AluOpType
DR = mybir.MatmulPerfMode.DoubleRow
bfnp = ml_dtypes.bfloat16
f8np = ml_dtypes.float8_e4m3

B, C, H, W = 8, 96, 64, 64
L = H * W
R, N, K = 6, 1, 4
EPS = 1e-5
P7 = 80          # padded row stride for the 7x7 conv (16-aligned for
                 # DoubleRow k-tile steps; cols 0:3 pad, 3:67 image, rest junk)
P7R = 70         # padded rows (3 + 64 + 3)
P7M = P7 * P7R + 16  # + margin for overrunning tap windows
P3 = 66          # padded width for 3x3 conv
NCHUNK = 8       # 4096 / 512
CH = 512
NB7 = 11         # 10 blocks of 6 rows + 1 block of 4 rows
NPAIR = 21       # 42 taps as vertical DoubleRow pairs + 7 singles (dh=6)

USE_DR = True    # fp8 DoubleRow dwconv7 (else plain fp8 per-tap matmuls)
USE_DMABC = True  # B/C row broadcast via 0-stride DMA (else PE matmul)

_CACHE = {}


def build_host_tensors(kw):
    """Precompute all weight/constant DRAM tensors (shared across cores)."""
    f = lambda a: np.asarray(a, np.float32)
    out = {}

    # --- fold LN gamma/beta into following 1x1 convs ---
    def fold(wname, bname, g, b):
        w = f(kw[wname])
        bb = f(kw[bname])
        return w * f(g)[None, :], bb + w @ f(b)

    fc1w, fc1b = fold("cn_fc1_w", "cn_fc1_b", kw["cn_ln_w"], kw["cn_ln_b"])
    ipw, ipb = fold("ip_w", "ip_b", kw["v_ln1_w"], kw["v_ln1_b"])
    opw, opb = fold("op_w", "op_b", kw["o_ln_w"], kw["o_ln_b"])
    mfc1w, mfc1b = fold("m_fc1_w", "m_fc1_b", kw["v_ln2_w"], kw["v_ln2_b"])
    fc2w, fc2b = f(kw["cn_fc2_w"]), f(kw["cn_fc2_b"])
    mfc2w, mfc2b = f(kw["m_fc2_w"]), f(kw["m_fc2_b"])

    # --- depthwise 7x7: fp8 DoubleRow tap-pair diagonals ---
    # per-channel power-of-2 scale so |w| lands in (0.25, 0.5]
    w7 = f(kw["cn_dw_w"]).reshape(C, 7, 7)
    m7 = np.maximum(np.abs(w7).reshape(C, 49).max(1), 1e-30)
    e7 = np.floor(np.log2(m7))
    s7 = np.exp2(-1.0 - e7)          # w*s in (0.25, 0.5]
    w7s = w7 * s7[:, None, None]
    ar = np.arange(C)
    # vertical pairs p = (dhp, dw): taps (2*dhp, dw) and (2*dhp+1, dw)
    # (delta = one padded row = 80 bytes, 16-aligned for DoubleRow)
    wdr = np.zeros((C, NPAIR * 2 * C), np.float32)
    for p in range(NPAIR):
        dhp, dw = p // 7, p % 7
        wdr[ar, p * 2 * C + ar] = w7s[:, 2 * dhp, dw]
        wdr[ar, p * 2 * C + C + ar] = w7s[:, 2 * dhp + 1, dw]
    out["wdr"] = wdr.astype(f8np)
    # dh=6 singles, one diag per dw
    wsing = np.zeros((C, 7 * C), np.float32)
    for dw in range(7):
        wsing[ar, dw * C + ar] = w7s[:, 6, dw]
    out["wsing"] = wsing.astype(f8np)

    # --- depthwise 3x3 diagonals (bf16, 9 taps) ---
    w3 = f(kw["dw_w"]).reshape(C, 9)
    diag3 = np.zeros((C, 9 * C), np.float32)
    for t in range(9):
        diag3[ar, t * C + ar] = w3[:, t]
    out["w3diag"] = diag3.astype(bfnp)

    # --- GEMM weights (lhsT layouts), bf16 ---
    out["wfc1"] = fc1w.T.astype(bfnp)  # [96, 384]
    wfc2 = np.zeros((128, 3 * C), np.float32)  # [128, 288] K-chunks
    for j in range(3):
        wfc2[:, j * C:(j + 1) * C] = fc2w[:, j * 128:(j + 1) * 128].T
    out["wfc2"] = wfc2.astype(bfnp)
    out["wip"] = ipw.T.astype(bfnp)  # [96, 96]
    out["wop"] = opw.T.astype(bfnp)
    out["wmfc1"] = mfc1w.T.astype(bfnp)
    wm2 = np.zeros((128, 3 * C), np.float32)
    for j in range(3):
        wm2[:, j * C:(j + 1) * C] = mfc2w[:, j * 128:(j + 1) * 128].T
    out["wmfc2"] = wm2.astype(bfnp)

    # x_proj lhsT [96, 32]: cols 0:8 k0, 8:16 k2, 16:24 k1, 24:32 k3
    xp = f(kw["x_proj_w"])  # [4, 8, 96]
    wxp = np.concatenate([xp[0].T, xp[2].T, xp[1].T, xp[3].T], axis=1)
    # negate the B rows (row 6 of each direction's 8-row block): the scan
    # needs delta*u*B = ln(dA)*u*(-B), which turns the bso op into a plain
    # TensorTensor (2x-capable) instead of a scalar_tensor_tensor
    wxp = wxp.copy()
    for c0 in range(4):
        wxp[:, c0 * 8 + 6] = -wxp[:, c0 * 8 + 6]
    out["wxp"] = wxp.astype(bfnp)  # [96, 32]

    # xdbl row bases within the [48, L] tile: k0@0, k2@8, k1@32, k3@40.
    dtw = f(kw["dt_w"])  # [4, 96, 6]
    wdt = np.zeros((48, 2 * C), np.float32)
    wdt[0:6, 0:C] = dtw[0].T
    wdt[32:38, 0:C] = dtw[1].T
    wdt[8:14, C:2 * C] = dtw[2].T
    wdt[40:46, C:2 * C] = dtw[3].T
    out["wdt"] = wdt.astype(bfnp)

    # ones for LN partition reduce / broadcast; identity for PSUM residual
    out["ones96"] = np.ones((C, C), np.float32)
    out["ones96_bf"] = np.ones((C, C), bfnp)
    out["i96"] = np.eye(C, dtype=np.float32)
    # row-broadcast selectors for the 8 (dir, B/C) rows of xdbl
    rows = [6, 38, 14, 46, 7, 39, 15, 47]  # B k0..k3, C k0..k3
    sel48 = np.zeros((48, 8 * C), np.float32)
    for i, r in enumerate(rows):
        sel48[r, i * C:(i + 1) * C] = 1.0
    out["sel48"] = sel48.astype(bfnp)

    # per-partition scalar bank [128, NV] fp32
    A = (-np.exp(f(kw["A_logs"]))).reshape(K, C)
    a_is_neg1 = bool(np.allclose(A, -1.0, atol=1e-6))
    Ds = f(kw["Ds"]).reshape(K, C)
    dtb = f(kw["dt_b"])  # [4, 96]
    cols = []

    def col(v, n=C):
        a = np.zeros(128, np.float32)
        a[: len(v)] = v
        cols.append(a)
        return len(cols) - 1

    ix = {}
    ix["cn_dw_b"] = col(f(kw["cn_dw_b"]))
    ix["descale7"] = col(1.0 / s7)
    for j in range(3):
        ix[f"fc1b{j}"] = col(fc1b[j * 128:(j + 1) * 128])
    ix["fc2b"] = col(fc2b)
    ix["ipb"] = col(ipb)
    ix["dwb"] = col(f(kw["dw_b"]))
    for k in range(4):
        ix[f"negdtb{k}"] = col(-dtb[k])
        ix[f"negA{k}"] = col(-A[k])
    ix["Dsum"] = col(Ds.sum(0))
    ix["eps"] = col(np.full(128, EPS, np.float32), 128)
    ix["opb"] = col(opb)
    for j in range(3):
        ix[f"mfc1b{j}"] = col(mfc1b[j * 128:(j + 1) * 128])
    ix["mfc2b"] = col(mfc2b)
    out["vecs"] = np.stack(cols, axis=1)  # [128, NV]
    return out, ix, a_is_neg1


def pad_image_fp8(x):
    """[96,64,64] fp32 -> padded [96, 70 rows x 80 cols + margin] fp8."""
    xp = np.zeros((C, P7R, P7), np.float32)
    xp[:, 3:3 + H, 3:3 + W] = x
    flat = np.zeros((C, P7M), np.float32)
    flat[:, :P7 * P7R] = xp.reshape(C, P7 * P7R)
    return flat.astype(f8np)


def r32(ap):
    return ap.bitcast(F32R)


def pair_view(ap2d, delta):
    """[P, n] AP -> [P, 2, n] AP with the pair dim at stride `delta`."""
    v = ap2d.unsqueeze(1).copy()
    v.ap[1] = [delta, 2]
    return v


def repeat_row(row_ap, n):
    """[1, L] AP -> [1, n, L] AP repeating the row via a 0-stride dim
    (DMA-legal: partition dim keeps its nonzero step)."""
    v = row_ap.unsqueeze(1).copy()
    v.ap[1] = [0, n]
    return v


def build_program(ix, a_is_neg1):
    nc = bacc.Bacc("TRN2", target_bir_lowering=False, debug=False)

    din = {}
    for name, shape, dt in [
        ("xpad", [C, P7M], FP8),
        ("xres", [C, L], F32R),
        ("wdr", [C, NPAIR * 2 * C], FP8),
        ("wsing", [C, 7 * C], FP8),
        ("w3diag", [C, 9 * C], BF16),
        ("wfc1", [C, 384], BF16),
        ("wfc2", [128, 3 * C], BF16),
        ("wip", [C, C], BF16),
        ("wop", [C, C], BF16),
        ("wmfc1", [C, 384], BF16),
        ("wmfc2", [128, 3 * C], BF16),
        ("wxp", [C, 32], BF16),
        ("wdt", [48, 2 * C], BF16),
    ] + ([("sel48", [48, 8 * C], BF16)] if not USE_DMABC else []) + [
        ("ones96", [C, C], F32R),
        ("ones96_bf", [C, C], BF16),
        ("i96", [C, C], F32R),
        ("vecs", [128, len(ix)], F32),
    ]:
        din[name] = nc.dram_tensor(name, shape, dt, kind="ExternalInput").ap()
    dout = nc.dram_tensor("out", [C, L], F32, kind="ExternalOutput").ap()

    class ActPhase:
        # Chains table-specific ACT ops in emission order so the scheduler
        # cannot interleave ops from different table sets (each set flip
        # costs an ACT function-table reload, ~1.3us).
        def __init__(self):
            self.cur_last = None

        def tag(self, bi):
            inst = bi.ins
            if self.cur_last is not None:
                add_dep_helper(inst, self.cur_last, sync=True,
                               reason="act table-set phase fence")
            self.cur_last = inst
            return bi

    ph = ActPhase()

    with tile.TileContext(nc) as tc:
        from contextlib import ExitStack

        with ExitStack() as ctx:
            const = ctx.enter_context(tc.tile_pool(name="const", bufs=1))
            bigp = ctx.enter_context(tc.tile_pool(name="big", bufs=1))
            scanp = ctx.enter_context(tc.tile_pool(name="scan", bufs=4))
            lnp = ctx.enter_context(tc.tile_pool(name="ln", bufs=2))
            hcp = ctx.enter_context(tc.tile_pool(name="hc", bufs=2))
            accp = ctx.enter_context(tc.tile_pool(name="acc", bufs=2))
            chk = ctx.enter_context(tc.tile_pool(name="chk", bufs=3))
            ps = ctx.enter_context(tc.tile_pool(name="ps", bufs=2, space="PSUM"))
            # PSUM banks: cv x2 + gen x2 + gen2 x2 + f1 x2 = 8
            psf1 = ctx.enter_context(
                tc.tile_pool(name="psf1", bufs=2, space="PSUM"))

            # ---- load constants ----
            cc = {}
            for name, ap in din.items():
                if name in ("xpad", "xres"):
                    continue
                t = const.tile(list(ap.shape), ap.dtype, tag=name, name=name)
                nc.sync.dma_start(t[:], ap)
                cc[name] = t
            # Route the bias bank through an ACT copy: the ACT instruction
            # encoding has a single sync-wait slot, so later ACT ops must not
            # need a DMA wait on top of their PSUM wait.
            nv = len(ix)
            vecs_sb = const.tile([128, nv], F32, tag="vecs_sb")
            nc.scalar.activation(vecs_sb[:], cc["vecs"][:], AF.Copy)
            scr = const.tile([128, 1], F32, tag="scr")
            nc.scalar.activation(scr[:], vecs_sb[:, 0:1], AF.Copy)
            V = lambda key: vecs_sb[:, ix[key]:ix[key] + 1]
            V96 = lambda key: vecs_sb[:C, ix[key]:ix[key] + 1]

            xpad = bigp.tile([C, P7M], FP8, tag="pad")
            nc.sync.dma_start(xpad[:], din["xpad"])
            xres = bigp.tile([C, L], F32R, tag="xres")
            nc.sync.dma_start(xres[:], din["xres"])

            # =============== helpers ===============
            def conv7_block(b):
                """fp8 DoubleRow dwconv7 for output rows [6b, 6b+nr)."""
                r0 = 6 * b
                nr = 6 if b < 10 else 4
                nflat = nr * P7
                pt = ps.tile([C, 480], F32, tag="cv", bufs=2, name="pt")
                base = lambda dh, dw: (r0 + dh) * P7 + dw
                for p in range(NPAIR):
                    dhp, dw = p // 7, p % 7
                    b0 = base(2 * dhp, dw)
                    if USE_DR:
                        # vertical tap pair: k-tile step = one padded row
                        # (80 B, 16-aligned as the DR ifmap AP requires)
                        rhs = pair_view(xpad[:, b0:b0 + nflat], P7)
                        w = cc["wdr"][:, p * 2 * C:(p + 1) * 2 * C].rearrange(
                            "c (t m) -> c t m", t=2)
                        nc.tensor.matmul(pt[:, 0:nflat], w, rhs,
                                         start=(p == 0), stop=False,
                                         perf_mode=DR)
                    else:
                        for t in range(2):
                            bt = base(2 * dhp + t, dw)
                            nc.tensor.matmul(
                                pt[:, 0:nflat],
                                cc["wdr"][:, (2 * p + t) * C:
                                          (2 * p + t + 1) * C],
                                xpad[:, bt:bt + nflat],
                                start=(p == 0 and t == 0), stop=False)
                for dw in range(7):
                    b6 = base(6, dw)
                    nc.tensor.matmul(pt[:, 0:nflat],
                                     cc["wsing"][:, dw * C:(dw + 1) * C],
                                     xpad[:, b6:b6 + nflat], start=False,
                                     stop=(dw == 6))
                # evacuate: descale + bias, drop pad columns (Pool cannot
                # read PSUM; ACT applies scale+bias in one op)
                src = pt[:, 0:nflat].rearrange("c (h w) -> c h w", w=P7)
                dst = hsb[:, r0 * W:(r0 + nr) * W].rearrange(
                    "c (h w) -> c h w", w=W)
                nc.scalar.activation(dst, src[:, :, 0:W], AF.Identity,
                                     scale=V96("descale7"),
                                     bias=V96("cn_dw_b"))

            def ln_chunk(src_chunk, src_f32, out_chunk, pool_xn=False):
                """out = (x - mean_c) / sqrt(var_c + eps) for one 512-chunk.

                Fused partition reduce+broadcast via all-ones [96,96] lhsT;
                rstd via ACT Sqrt (single-function table phase) + DVE
                fast reciprocal.
                """
                mb = ps.tile([C, CH], F32, tag="gen", name="mb")
                if src_f32:
                    nc.tensor.matmul(mb[:], cc["ones96"][:], r32(src_chunk),
                                     start=True, stop=True)
                else:
                    nc.tensor.matmul(mb[:], cc["ones96_bf"][:], src_chunk,
                                     start=True, stop=True)
                d = chk.tile([C, CH], BF16, tag="lnd", name="d", bufs=4)
                nc.vector.scalar_tensor_tensor(d[:], mb[:], -1.0 / C,
                                               src_chunk, OP.mult, OP.add)
                dsq = chk.tile([C, CH], BF16, tag="sq", name="dsq", bufs=4)
                nc.gpsimd.tensor_tensor(dsq[:], d[:], d[:], OP.mult)
                vb = ps.tile([C, CH], F32, tag="gen2", name="vb")
                nc.tensor.matmul(vb[:], cc["ones96_bf"][:], dsq[:],
                                 start=True, stop=True)
                sd = chk.tile([C, CH], F32, tag="sd", bufs=2, name="sd")
                ph.tag(nc.scalar.activation(sd[:], vb[:], AF.Sqrt,
                                            scale=1.0 / C, bias=V96("eps")))
                rs = chk.tile([C, CH], F32, tag="rs", bufs=2, name="rs")
                nc.vector.reciprocal_approx_fast(out=rs[:], in_=sd[:])
                eng = nc.gpsimd if pool_xn else nc.vector
                eng.tensor_tensor(out_chunk, d[:], rs[:], OP.mult)

            def mlp_tail(j, wf1, wf2, b1pfx, b2key, res_tile, out_tile, xnf):
                """chunk j: fc1 -> gelu -> fc2 (+res via PSUM identity mm)."""
                xn = xnf[:, j * CH:(j + 1) * CH]
                gs = []
                for mm in range(3):
                    f1 = psf1.tile([128, CH], F32, tag="f1", name=f"f1_{mm}")
                    nc.tensor.matmul(f1[:], cc[wf1][:, mm * 128:(mm + 1) * 128],
                                     xn, start=True, stop=True)
                    g = chk.tile([128, CH], BF16, tag=f"g{mm}", name=f"g{mm}", bufs=2)
                    ph.tag(nc.scalar.activation(g[:], f1[:], AF.Gelu,
                                                bias=V(f"{b1pfx}{mm}")))
                    gs.append(g)
                f2 = ps.tile([C, CH], F32, tag="gen2", name="f2", bufs=2)
                nc.tensor.matmul(f2[:], cc["i96"][:],
                                 r32(res_tile[:, j * CH:(j + 1) * CH]),
                                 start=True, stop=False)
                for mm in range(3):
                    nc.tensor.matmul(f2[:], cc[wf2][:, mm * C:(mm + 1) * C],
                                     gs[mm][:], start=False, stop=(mm == 2))
                nc.vector.tensor_scalar(out_tile[:, j * CH:(j + 1) * CH]
                                        .bitcast(F32R), f2[:], V96(b2key),
                                        None, OP.add)

            # =============== ConvNeXt block (interleaved with dwconv7) ======
            hsb = bigp.tile([C, L], BF16, tag="bufA", name="hsb")
            xnf_cn = lnp.tile([C, L], BF16, tag="ln", name="xnf_cn")
            nb = 0
            for j in range(NCHUNK):
                need = min(NB7 - 1, (8 * j + 7) // 6)
                while nb <= need:
                    conv7_block(nb)
                    nb += 1
                ln_chunk(hsb[:, j * CH:(j + 1) * CH], False,
                         xnf_cn[:, j * CH:(j + 1) * CH], pool_xn=True)
            while nb < NB7:
                conv7_block(nb)
                nb += 1
            x1 = bigp.tile([C, L], F32, tag="x1", name="x1")
            for j in range(NCHUNK):
                mlp_tail(j, "wfc1", "wfc2", "fc1b", "fc2b", xres, x1, xnf_cn)

            # =============== SS2D: LN1 + in_proj + dwconv3 + silu ==========
            xn1f = lnp.tile([C, L], BF16, tag="ln", name="xn1f")
            for j in range(NCHUNK):
                ln_chunk(x1[:, j * CH:(j + 1) * CH], True,
                         xn1f[:, j * CH:(j + 1) * CH], pool_xn=False)
            v2pad = bigp.tile([C, P3 * P3], BF16, tag="pad2", name="v2pad")
            nc.gpsimd.memset(v2pad[:], 0.0)
            v2int = v2pad[:].rearrange("c (h w) -> c h w", w=P3)
            for j in range(NCHUNK):
                pv = ps.tile([C, CH], F32, tag="gen", name="pv")
                nc.tensor.matmul(pv[:], cc["wip"][:],
                                 xn1f[:, j * CH:(j + 1) * CH], start=True,
                                 stop=True)
                dst = v2int[:, 1 + j * 8:1 + (j + 1) * 8, 1:1 + W]
                nc.vector.tensor_scalar(dst, pv[:], V96("ipb"), None, OP.add)
            # dwconv3 (bf16) + silu via sigmoid
            v4 = bigp.tile([C, L], BF16, tag="bufA", name="v4")
            v2src = v2pad[:].rearrange("c (h w) -> c h w", w=P3)
            for j in range(NCHUNK):
                pc = ps.tile([C, CH], F32, tag="cv", bufs=2, name="pc")
                r0 = j * 8
                for t in range(9):
                    dh, dw = t // 3, t % 3
                    rhs = v2src[:, r0 + dh:r0 + dh + 8, dw:dw + W]
                    nc.tensor.matmul(pc[:],
                                     cc["w3diag"][:, t * C:(t + 1) * C],
                                     rhs, start=(t == 0), stop=(t == 8))
                sg = chk.tile([C, CH], BF16, tag="lnd", name="sg", bufs=4)
                ph.tag(nc.scalar.activation(sg[:], pc[:], AF.Sigmoid,
                                            bias=V96("dwb")))
                xb = chk.tile([C, CH], BF16, tag="sq", name="xb", bufs=4)
                nc.vector.tensor_scalar(xb[:], pc[:], V96("dwb"), None,
                                        OP.add)
                nc.gpsimd.tensor_tensor(v4[:, j * CH:(j + 1) * CH], xb[:],
                                        sg[:], OP.mult)

            # =============== cross-scan projections ===============
            # xdbl [48, 4096]: rows 0:8 k0, 8:16 k2 (l-major); 32:40 k1,
            # 40:48 k3 (w-major).
            v4T = v4[:].rearrange("c (h w) -> c h w", w=W).transpose([0, 2, 1])
            xdbl = bigp.tile([48, L], BF16, tag="xdbl", name="xdbl")
            for j in range(NCHUNK):
                p1 = ps.tile([48, CH], F32, tag="cv", bufs=2, name="p1")
                nc.tensor.matmul(p1[0:16, :], cc["wxp"][:, 0:16],
                                 v4[:, j * CH:(j + 1) * CH], start=True,
                                 stop=True)
                rhsT = v4T[:, j * 8:(j + 1) * 8, :]
                nc.tensor.matmul(p1[32:48, :], cc["wxp"][:, 16:32], rhsT,
                                 start=True, stop=True)
                nc.vector.tensor_copy(xdbl[:, j * CH:(j + 1) * CH], p1[:])

            # =============== per-direction scan ===============
            # Phase 1: all dt-projections + sigmoid evacuations (one table set)
            dAs = {}
            for k in [0, 2, 1, 3]:
                blk0 = 0 if k in (0, 2) else 32
                dtcol = 0 if k in (0, 1) else 1
                dA = scanp.tile([C, L], BF16, tag="dA", bufs=4, name=f"dA{k}")
                for j in range(NCHUNK):
                    pd = ps.tile([C, CH], F32, tag="gen", name="pd")
                    nc.tensor.matmul(
                        pd[:],
                        cc["wdt"][blk0:blk0 + 16, dtcol * C:(dtcol + 1) * C],
                        xdbl[blk0:blk0 + 16, j * CH:(j + 1) * CH],
                        start=True, stop=True)
                    ph.tag(nc.scalar.activation(
                        dA[:, j * CH:(j + 1) * CH], pd[:], AF.Sigmoid,
                        scale=-1.0, bias=V96(f"negdtb{k}")))
                dAs[k] = dA
            # Phase 2+3: per-direction chains, full-tile. B/C rows of xdbl are
            # partition-broadcast on the Pool engine (d_state=1 so they are
            # single rows); the products run full-tile on DVE.
            brow = {0: 6, 1: 38, 2: 14, 3: 46}
            crow = {0: 7, 1: 39, 2: 15, 3: 47}
            def bcast_row(dst, row, selcol):
                """broadcast one xdbl row into a [C, L] tile."""
                if USE_DMABC:
                    nc.sync.dma_start(dst[:],
                                      repeat_row(xdbl[row:row + 1, :], C))
                else:
                    for j in range(NCHUNK):
                        bp = ps.tile([C, CH], F32, tag="gen2", name="bp", bufs=2)
                        nc.tensor.matmul(
                            bp[:], cc["sel48"][:, selcol * C:(selcol + 1) * C],
                            xdbl[:, j * CH:(j + 1) * CH], start=True,
                            stop=True)
                        nc.vector.tensor_copy(dst[:, j * CH:(j + 1) * CH],
                                              bp[:])

            ys = {}
            for k in [0, 2, 1, 3]:
                uview = v4[:] if k in (0, 2) else v4T
                bb = scanp.tile([C, L], BF16, tag="w", bufs=4, name=f"bb{k}")
                bcast_row(bb, brow[k], [6, 38, 14, 46].index(brow[k]))
                # delta = -ln(dA) (full tile, natural_log set)
                ld = scanp.tile([C, L], BF16, tag="w", bufs=4, name=f"ld{k}")
                ph.tag(nc.scalar.activation(ld[:], dAs[k][:], AF.Ln))
                if not a_is_neg1:
                    # general A: dA = exp(A*delta) = exp((-A)*ld)
                    ph.tag(nc.scalar.activation(dAs[k][:], ld[:], AF.Exp,
                                                scale=V96(f"negA{k}")))
                # uB = u * (-B)_bcast (Pool: prefetches across directions
                # while DVE runs the serial chains); bso = ld * uB
                uB = scanp.tile([C, L], BF16, tag="w", bufs=4, name=f"uB{k}")
                if k in (0, 2):
                    nc.gpsimd.tensor_tensor(uB[:], bb[:], uview, OP.mult)
                else:
                    # 3D transposed view: keep on DVE (Pool is 2.2x slower
                    # and this op is harder to hide)
                    nc.vector.tensor_tensor(uB[:], bb[:], uview, OP.mult)
                bso = scanp.tile([C, L], BF16, tag="w", bufs=4,
                                 name=f"bso{k}")
                nc.vector.tensor_tensor(bso[:], ld[:], uB[:], OP.mult)
                # h = scan(dA, bso)
                h = scanp.tile([C, L], BF16, tag="w", bufs=4, name=f"h{k}")
                if k in (0, 1):
                    nc.vector.tensor_tensor_scan(h[:], dAs[k][:], bso[:], 0.0,
                                                 OP.mult, OP.add)
                else:
                    nc.vector.tensor_tensor_scan(h[:][:, ::-1],
                                                 dAs[k][:][:, ::-1],
                                                 bso[:][:, ::-1], 0.0,
                                                 OP.mult, OP.add)
                # y_k = h * C_bcast
                cb = scanp.tile([C, L], BF16, tag="w", bufs=4, name=f"cb{k}")
                bcast_row(cb, crow[k], 4 + [7, 39, 15, 47].index(crow[k]))
                if k in (0, 1):
                    y = hcp.tile([C, L], BF16, tag="hc", name=f"y{k}")
                else:
                    y = accp.tile([C, L], BF16, tag="acc", name=f"y{k}")
                nc.vector.tensor_tensor(y[:], h[:], cb[:], OP.mult)
                ys[k] = y

            # =============== cross-merge + D*u + LN + out_proj =============
            a1 = scanp.tile([C, L], BF16, tag="dA", bufs=4, name="a1")
            nc.vector.tensor_tensor(a1[:], ys[0][:], ys[2][:], OP.add)
            a2 = scanp.tile([C, L], BF16, tag="dA", bufs=4, name="a2")
            nc.vector.tensor_tensor(a2[:], ys[1][:], ys[3][:], OP.add)
            a2T = a2[:].rearrange("c (w h) -> c w h", w=W).transpose(
                [0, 2, 1])
            t2 = scanp.tile([C, L], BF16, tag="dA", bufs=4, name="t2")
            nc.vector.tensor_tensor(t2[:], a1[:], a2T, OP.add)
            preln = scanp.tile([C, L], BF16, tag="dA", bufs=4, name="preln")
            nc.vector.scalar_tensor_tensor(preln[:], v4[:], V96("Dsum"),
                                           t2[:], OP.mult, OP.add)
            ynf = lnp.tile([C, L], BF16, tag="ln", name="ynf")
            for j in range(NCHUNK):
                ln_chunk(preln[:, j * CH:(j + 1) * CH], False,
                         ynf[:, j * CH:(j + 1) * CH], pool_xn=False)
            x2 = bigp.tile([C, L], F32, tag="xres", name="x2")
            for j in range(NCHUNK):
                po = ps.tile([C, CH], F32, tag="gen", name="po")
                nc.tensor.matmul(po[:], cc["i96"][:],
                                 r32(x1[:, j * CH:(j + 1) * CH]),
                                 start=True, stop=False)
                nc.tensor.matmul(po[:], cc["wop"][:],
                                 ynf[:, j * CH:(j + 1) * CH], start=False,
                                 stop=True)
                nc.scalar.activation(x2[:, j * CH:(j + 1) * CH]
                                     .bitcast(F32R), po[:],
                                     AF.Identity, bias=V96("opb"))

            # =============== MLP block ===============
            xnf_m = lnp.tile([C, L], BF16, tag="ln", name="xnf_m")
            for j in range(NCHUNK):
                ln_chunk(x2[:, j * CH:(j + 1) * CH], True,
                         xnf_m[:, j * CH:(j + 1) * CH], pool_xn=True)
            outsb = bigp.tile([C, L], F32, tag="x1", name="outsb")
            for j in range(NCHUNK):
                mlp_tail(j, "wmfc1", "wmfc2", "mfc1b", "mfc2b", x2, outsb,
                         xnf_m)
            nc.sync.dma_start(dout, outsb[:])

    nc.compile()
    return nc


def get_program_and_inputs(inputs):
    host, ix, a_is_neg1 = build_host_tensors(inputs)
    key = ("prog", a_is_neg1)
    if key not in _CACHE:
        _CACHE[key] = build_program(ix, a_is_neg1)
    nc = _CACHE[key]
    x = np.asarray(inputs["x"], np.float32)
    in_maps = []
    for b in range(B):
        m = {k: v for k, v in host.items()}
        m["xpad"] = pad_image_fp8(x[b])
        m["xres"] = x[b].reshape(C, L).astype(np.float32)
        in_maps.append(m)
    return nc, in_maps


def kernel(**inputs):
    nc, in_maps = get_program_and_inputs(inputs)
    res = run_bass_kernel_spmd(nc, in_maps, list(range(B)))
    out = np.stack([res.results[b]["out"].reshape(C, H, W) for b in range(B)])
    return out.astype(np.float32)


if __name__ == "__main__":
    # smoke build
    shapes = [
        ("x", (B, C, H, W)), ("cn_dw_w", (C, 7, 7)), ("cn_dw_b", (C,)),
        ("cn_ln_w", (C,)), ("cn_ln_b", (C,)), ("cn_fc1_w", (4 * C, C)),
        ("cn_fc1_b", (4 * C,)), ("cn_fc2_w", (C, 4 * C)), ("cn_fc2_b", (C,)),
        ("v_ln1_w", (C,)), ("v_ln1_b", (C,)), ("ip_w", (C, C)),
        ("ip_b", (C,)), ("dw_w", (C, 3, 3)), ("dw_b", (C,)),
        ("x_proj_w", (K, R + 2 * N, C)), ("dt_w", (K, C, R)),
        ("dt_b", (K, C)), ("A_logs", (K * C, N)), ("Ds", (K * C,)),
        ("o_ln_w", (C,)), ("o_ln_b", (C,)), ("op_w", (C, C)),
        ("op_b", (C,)), ("v_ln2_w", (C,)), ("v_ln2_b", (C,)),
        ("m_fc1_w", (4 * C, C)), ("m_fc1_b", (4 * C,)),
        ("m_fc2_w", (C, 4 * C)), ("m_fc2_b", (C,)),
    ]
    rng = np.random.default_rng(0)
    dummy = {k: rng.standard_normal(s).astype(np.float32) * 0.02
             for k, s in shapes}
    host, ix, neg1 = build_host_tensors(dummy)
    nc = build_program(ix, neg1)
    print("program built OK:", len(list(nc.all_instructions())),
          "instructions")
